# revision 34
# baseline (speedup 1.0000x reference)
"""Balanced supervised contrastive regression loss on 8 trn2 cores.

Math: rows of P are unit-norm so rowmax(P@P.T)=1 and the reference's
E = exp((P@P.T - 1)/T) + 1e-5. With tw_i = weights[t_i-40],
A = E * tw_i * tw_j, v = t_i - 40 (121 label values):
  denom[i,j] = tw_j * sum_u Gp[j,u] * [|v-u| >= |v-lbl_j|]
  Gp[j,u]    = sum_{k: lbl_k=u} E[j,k] * tw_k  = (E0 @ OHW)[j,u] + 1e-5*s_u
where E0 = exp((P@P.T-1)/T) (no epsilon), OHW[k,u] = tw_k*[lbl_k=u],
s_u = sum_{lbl_k=u} tw_k.  The per-anchor log-sums sum_j ln E[i,j] enter the
loss only via their GLOBAL sum (constant divisor N-1), so the device only
needs SUMLNE = sum_ij ln E[i,j].

Device (per core c, R=256 anchor columns; bf16 matmuls, fp32 PSUM):
  loop over 4 quads (4 k-chunks of 128 each):
    lt[128k, 4*256] += pt_d^T ptr_d   (16 MMs into one 4-bank PSUM tile)
    et = Exp(lt/T - 1/T)              (one ACT op per quad, bf16 out)
    Ln(et + 1e-5, accum_out=sacc[:,q]) (one ACT op; only the free-dim sum kept)
    gacc[121,256] += ohwT_kc^T et_kc  (4 MMs, lagged one quad for pipelining)
  sl[1,4] = ones^T sacc               (single final MM)
Host: +1e-5 corrections, tiny [2048,121] label-space loop, scalar assembly.
"""
import numpy as np
import ml_dtypes

N, D, VOCAB, OFF = 2048, 512, 121, 40
TEMP = 0.07
NCORES = 8
R = N // NCORES  # 256 anchor columns per core
KC = N // 128    # 16 k chunks
DC = D // 128    # 4 d chunks
NQ = 4           # chunks per quad
QN = KC // NQ    # number of quads

TRACE = False
TRACE_CORES = None
LAST = None  # BassKernelResults of the most recent device run

# processing units (number of 128-k-chunks each); quads steady-state,
# pairs at the edges for finer pipeline ramp/drain
UNITS = (4, 4, 4, 2, 2)

BF16 = ml_dtypes.bfloat16


def _build_nc():
    import concourse.bass as bass
    import concourse.mybir as mybir
    from concourse import tile

    f32 = mybir.dt.float32
    bf16 = mybir.dt.bfloat16
    AF = mybir.ActivationFunctionType
    nc = bass.Bass()

    # per-quad pack: [4 d-blocks x 512 pt cols | 4 chunks x 121 ohw cols]
    PKW = NQ * 128  # 512 pt columns per quad
    PKO = NQ * VOCAB  # 484 ohw columns per quad
    pk_d = [
        nc.declare_dram_parameter(f"pk{q}", [128, DC * PKW + PKO], bf16,
                                  isOutput=False)
        for q in range(QN)
    ]
    starts = np.cumsum([0] + list(UNITS))
    assert starts[-1] == KC
    units = [(int(starts[i]), int(UNITS[i])) for i in range(len(UNITS))]
    NU = len(units)

    bias_d = nc.declare_dram_parameter("biasv", [128, 3], f32, isOutput=False)
    # single output: [:VOCAB, :R] = gacc copy, [:, R:R+NU] = sacc
    out_d = nc.declare_dram_parameter("outt", [128, R + NU], f32,
                                      isOutput=True)

    inv_t = 1.0 / TEMP

    with tile.TileContext(nc) as tc:
        with (
            tc.tile_pool(name="const", bufs=1) as cpool,
            tc.tile_pool(name="work", bufs=3) as wpool,
            tc.tile_pool(name="psq", bufs=2, space="PSUM") as psq,
            tc.tile_pool(name="psp", bufs=2, space="PSUM") as psp,
            tc.tile_pool(name="acc", bufs=1, space="PSUM") as apool,
        ):
            # pack 0 split per d-block so the first matmuls start sooner
            pk0a = cpool.tile([128, PKW], bf16, tag="pk0a")
            nc.sync.dma_start(pk0a[:], pk_d[0][:, 0:PKW])
            pk0b = cpool.tile([128, PKW], bf16, tag="pk0b")
            nc.sync.dma_start(pk0b[:], pk_d[0][:, PKW:2 * PKW])
            bias_t = cpool.tile([128, 3], f32, tag="biasv")
            nc.sync.dma_start(bias_t[:], bias_d[:])
            pk0c = cpool.tile([128, 2 * PKW + PKO], bf16, tag="pk0c")
            nc.sync.dma_start(pk0c[:], pk_d[0][:, 2 * PKW:])
            packs = [None]
            for q in range(1, QN):
                t = cpool.tile([128, DC * PKW + PKO], bf16, tag=f"pk{q}")
                nc.sync.dma_start(t[:], pk_d[q][:])
                packs.append(t)

            def pk(q, col0, width):
                """AP into pack q's columns [col0, col0+width)."""
                if q == 0:
                    if col0 + width <= PKW:
                        return pk0a[:, col0:col0 + width]
                    if col0 + width <= 2 * PKW:
                        return pk0b[:, col0 - PKW:col0 - PKW + width]
                    return pk0c[:, col0 - 2 * PKW:col0 - 2 * PKW + width]
                return packs[q][:, col0:col0 + width]

            # this core's anchor columns: first R cols of each d-block of pk 0
            ptr_t = [pk(0, d * PKW, R) for d in range(DC)]

            outt = cpool.tile([128, R + NU], f32, tag="outt")
            # rows VOCAB..127 of the gacc region are never written; zero them
            # (32-aligned base; rows 96..VOCAB are re-written by the copy)
            nc.gpsimd.memset(outt[96:128, 0:R], 0.0)
            gacc = apool.tile([VOCAB, R], f32, tag="gacc")

            def g_mms(ui):
                ck0, nch = units[ui]
                for c in range(nch):
                    kc = ck0 + c
                    q, cq = kc // NQ, kc % NQ
                    nc.tensor.matmul(
                        gacc[:],
                        pk(q, DC * PKW + cq * VOCAB, VOCAB),
                        et_u[ui][:, c * R:(c + 1) * R],
                        start=(kc == 0),
                        stop=(kc == KC - 1),
                    )

            def ln_op(ui):
                lg = wpool.tile([128, units[ui][1] * R], bf16,
                                tag="lgq" if units[ui][1] == NQ else "lgp")
                nc.scalar.activation(lg[:], et_u[ui][:], AF.Ln,
                                     bias=bias_t[:, 1:2],
                                     accum_out=outt[:, R + ui:R + ui + 1])

            et_u = {}
            for ui, (ck0, nch) in enumerate(units):
                pool = psq if nch == NQ else psp
                lt = pool.tile([128, nch * R], f32,
                               tag="ltq" if nch == NQ else "ltp")
                for c in range(nch):
                    kc = ck0 + c
                    q, cq = kc // NQ, kc % NQ
                    for d in range(DC):
                        nc.tensor.matmul(
                            lt[:, c * R:(c + 1) * R],
                            pk(q, d * PKW + cq * 128, 128),
                            ptr_t[d],
                            start=(d == 0),
                            stop=(d == DC - 1),
                        )
                et = wpool.tile([128, nch * R], bf16,
                                tag="etq" if nch == NQ else "etp")
                nc.scalar.activation(et[:], lt[:], AF.Exp,
                                     bias=bias_t[:, 0:1], scale=inv_t)
                et_u[ui] = et
                if ui >= 1:
                    ln_op(ui - 1)
                    g_mms(ui - 1)
            ln_op(NU - 1)
            g_mms(NU - 1)

            nc.vector.tensor_copy(outt[:VOCAB, :R], gacc[:])
            nc.sync.dma_start(out_d[:], outt[:])
    return nc


def _device_run(P, ohw):
    """Returns Gp0 [N,121] = E0@OHW and SUMLNE = sum_ij ln(E0+1e-5)."""
    from concourse.bass_utils import run_bass_kernel_spmd

    nc = _build_nc()
    PKW = NQ * 128
    PT = np.ascontiguousarray(P.T).astype(BF16)  # [D, N]
    ohwb = ohw.astype(BF16)                      # [N, VOCAB]
    inv_t = 1.0 / TEMP
    biasv = np.empty((128, 3), np.float32)
    biasv[:, 0] = -inv_t
    biasv[:, 1] = 1e-5
    biasv[:, 2] = 1.0

    def make_pack(kchunks):
        """kchunks: NQ global 128-k-chunk indices, in processing order."""
        pk = np.empty((128, DC * PKW + NQ * VOCAB), BF16)
        for ci, kc in enumerate(kchunks):
            for d in range(DC):
                pk[:, d * PKW + ci * 128:d * PKW + (ci + 1) * 128] = \
                    PT[d * 128:(d + 1) * 128, kc * 128:(kc + 1) * 128]
            pk[:, DC * PKW + ci * VOCAB:DC * PKW + (ci + 1) * VOCAB] = \
                ohwb[kc * 128:(kc + 1) * 128, :]
        return pk

    in_maps = []
    for c in range(NCORES):
        qc, off = c // 2, (c % 2) * 2  # own quad; chunk rotation (2 chunks)
        own = [qc * NQ + ((ci + off) % NQ) for ci in range(NQ)]
        quads = [own] + [[q * NQ + ci for ci in range(NQ)]
                         for q in range(QN) if q != qc]
        m = {f"pk{q}": make_pack(quads[q]) for q in range(QN)}
        m["biasv"] = biasv
        in_maps.append(m)
    res = run_bass_kernel_spmd(
        nc, in_maps, list(range(NCORES)),
        trace=TRACE,
        trace_cores=TRACE_CORES if TRACE else None,
    )
    globals()["LAST"] = res
    r = res.results
    Gp0 = np.concatenate(
        [np.asarray(x["outt"], np.float32)[:VOCAB, :R].T for x in r], 0)
    sumlne = float(sum(
        np.asarray(x["outt"], np.float64)[:, R:].sum() for x in r))
    return Gp0.astype(np.float64), sumlne


def _host_fallback(P, ohw):
    L = (P @ P.T - 1.0) / TEMP
    E0 = np.exp(L)
    return E0 @ ohw, float(np.log(E0 + 1e-5).sum())


def kernel(projections, targets, weights):
    P = np.asarray(projections, np.float32)
    t = np.asarray(targets).astype(np.int64)
    w = np.asarray(weights, np.float32)
    lbl = t - OFF
    tw = w[lbl].astype(np.float32)
    ohw = np.zeros((N, VOCAB), np.float32)
    ohw[np.arange(N), lbl] = tw

    try:
        Gp0, sumlne = _device_run(P, ohw)
    except Exception as e:  # pragma: no cover - safety net
        import traceback
        traceback.print_exc()
        print("DEVICE PATH FAILED - host fallback:", e)
        Gp0, sumlne = _host_fallback(P.astype(np.float64), ohw.astype(np.float64))

    twd = tw.astype(np.float64)
    s_u = np.bincount(lbl, weights=twd, minlength=VOCAB)
    Gp = Gp0 + 1e-5 * s_u[None, :]          # [N,VOCAB]
    G = twd[:, None] * Gp
    rowsumA = twd * Gp.sum(1)               # denom diagonal
    u = np.arange(VOCAB)
    Cm = np.abs(u[:, None] - u[None, :])    # [v,u]
    Bv = np.abs(u[:, None] - lbl[None, :])  # [v,j]
    LDsum = np.empty(VOCAB, np.float64)
    for v in range(VOCAB):
        M = Cm[v][None, :] >= Bv[v][:, None]  # [j,u]
        Dv = (G * M).sum(1)
        LDsum[v] = np.log(Dv).sum()
    ltw = np.log(twd)
    SLT = ltw.sum()
    num = (LDsum[lbl].sum() - np.log(rowsumA).sum()
           - sumlne - 2.0 * N * SLT
           + N * np.log1p(1e-5) + 2.0 * SLT)
    return np.float32(num / (N * (N - 1 + 1e-5)))


# revision 38
# speedup vs baseline: 1.0257x; 1.0257x over previous
"""Balanced supervised contrastive regression loss on 8 trn2 cores.

Math: rows of P are unit-norm so rowmax(P@P.T)=1 and the reference's
E = exp((P@P.T - 1)/T) + 1e-5. With tw_i = weights[t_i-40],
A = E * tw_i * tw_j, v = t_i - 40 (121 label values):
  denom[i,j] = tw_j * sum_u Gp[j,u] * [|v-u| >= |v-lbl_j|]
  Gp[j,u]    = sum_{k: lbl_k=u} E[j,k] * tw_k  = (E0 @ OHW)[j,u] + 1e-5*s_u
where E0 = exp((P@P.T-1)/T) (no epsilon), OHW[k,u] = tw_k*[lbl_k=u],
s_u = sum_{lbl_k=u} tw_k.  The per-anchor log-sums sum_j ln E[i,j] enter the
loss only via their GLOBAL sum (constant divisor N-1), so the device only
needs SUMLNE = sum_ij ln E[i,j].

Device (per core c, R=256 anchor columns; bf16 matmuls, fp32 PSUM):
  loop over 4 quads (4 k-chunks of 128 each):
    lt[128k, 4*256] += pt_d^T ptr_d   (16 MMs into one 4-bank PSUM tile)
    et = Exp(lt/T - 1/T)              (one ACT op per quad, bf16 out)
    Ln(et + 1e-5, accum_out=sacc[:,q]) (one ACT op; only the free-dim sum kept)
    gacc[121,256] += ohwT_kc^T et_kc  (4 MMs, lagged one quad for pipelining)
  sl[1,4] = ones^T sacc               (single final MM)
Host: +1e-5 corrections, tiny [2048,121] label-space loop, scalar assembly.
"""
import numpy as np
import ml_dtypes

N, D, VOCAB, OFF = 2048, 512, 121, 40
TEMP = 0.07
NCORES = 8
R = N // NCORES  # 256 anchor columns per core
KC = N // 128    # 16 k chunks
DC = D // 128    # 4 d chunks
NQ = 4           # chunks per quad
QN = KC // NQ    # number of quads

TRACE = False
TRACE_CORES = None
LAST = None  # BassKernelResults of the most recent device run

# processing units (number of 128-k-chunks each); quads steady-state,
# pairs at the edges for finer pipeline ramp/drain
UNITS = (4, 4, 4, 2, 2)

BF16 = ml_dtypes.bfloat16


def _build_nc():
    import concourse.bass as bass
    import concourse.mybir as mybir
    from concourse import tile

    f32 = mybir.dt.float32
    bf16 = mybir.dt.bfloat16
    AF = mybir.ActivationFunctionType
    nc = bass.Bass()

    # per-quad pack: [4 d-blocks x 512 pt cols | 4 chunks x 121 ohw cols]
    PKW = NQ * 128  # 512 pt columns per quad
    PKO = NQ * VOCAB  # 484 ohw columns per quad
    pk_d = [
        nc.declare_dram_parameter(f"pk{q}", [128, DC * PKW + PKO], bf16,
                                  isOutput=False)
        for q in range(QN)
    ]
    starts = np.cumsum([0] + list(UNITS))
    assert starts[-1] == KC
    units = [(int(starts[i]), int(UNITS[i])) for i in range(len(UNITS))]
    NU = len(units)

    bias_d = nc.declare_dram_parameter("biasv", [128, 3], f32, isOutput=False)
    # single output: [:VOCAB, :R] = gacc copy, [:, R:R+NU] = sacc
    out_d = nc.declare_dram_parameter("outt", [128, R + NU], f32,
                                      isOutput=True)

    inv_t = 1.0 / TEMP

    with tile.TileContext(nc) as tc:
        with (
            tc.tile_pool(name="const", bufs=1) as cpool,
            tc.tile_pool(name="work", bufs=3) as wpool,
            tc.tile_pool(name="psq", bufs=2, space="PSUM") as psq,
            tc.tile_pool(name="psp", bufs=2, space="PSUM") as psp,
            tc.tile_pool(name="acc", bufs=1, space="PSUM") as apool,
        ):
            # pack 0 split per d-block so the first matmuls start sooner
            pk0a = cpool.tile([128, PKW], bf16, tag="pk0a")
            nc.sync.dma_start(pk0a[:], pk_d[0][:, 0:PKW])
            pk0b = cpool.tile([128, PKW], bf16, tag="pk0b")
            nc.sync.dma_start(pk0b[:], pk_d[0][:, PKW:2 * PKW])
            bias_t = cpool.tile([128, 3], f32, tag="biasv")
            nc.sync.dma_start(bias_t[:], bias_d[:])
            pk0c = cpool.tile([128, 2 * PKW + PKO], bf16, tag="pk0c")
            nc.sync.dma_start(pk0c[:], pk_d[0][:, 2 * PKW:])
            packs = [None]
            for q in range(1, QN):
                t = cpool.tile([128, DC * PKW + PKO], bf16, tag=f"pk{q}")
                nc.sync.dma_start(t[:], pk_d[q][:])
                packs.append(t)

            def pk(q, col0, width):
                """AP into pack q's columns [col0, col0+width)."""
                if q == 0:
                    if col0 + width <= PKW:
                        return pk0a[:, col0:col0 + width]
                    if col0 + width <= 2 * PKW:
                        return pk0b[:, col0 - PKW:col0 - PKW + width]
                    return pk0c[:, col0 - 2 * PKW:col0 - 2 * PKW + width]
                return packs[q][:, col0:col0 + width]

            # this core's anchor columns: first R cols of each d-block of pk 0
            ptr_t = [pk(0, d * PKW, R) for d in range(DC)]

            outt = cpool.tile([128, R + NU], f32, tag="outt")
            # padded to 128 rows; the matmuls write [:VOCAB], the copy ships
            # all 128 (host ignores rows >= VOCAB)
            gacc = apool.tile([128, R], f32, tag="gacc")

            # fold the bias-DMA dependency into the ACT engine's clock once,
            # so each activation below needs only its PE wait (the ACT ISA
            # slot carries a single sync wait)
            scr = cpool.tile([128, 1], f32, tag="scr")
            nc.scalar.copy(scr[:], bias_t[:, 0:1])

            def g_mms(ui):
                ck0, nch = units[ui]
                for c in range(nch):
                    kc = ck0 + c
                    q, cq = kc // NQ, kc % NQ
                    nc.tensor.matmul(
                        gacc[:VOCAB, :],
                        pk(q, DC * PKW + cq * VOCAB, VOCAB),
                        et_u[ui][:, c * R:(c + 1) * R],
                        start=(kc == 0),
                        stop=(kc == KC - 1),
                    )

            def ln_op(ui):
                lg = wpool.tile([128, units[ui][1] * R], bf16,
                                tag="lgq" if units[ui][1] == NQ else "lgp")
                nc.scalar.activation(lg[:], et_u[ui][:], AF.Ln,
                                     bias=bias_t[:, 1:2],
                                     accum_out=outt[:, R + ui:R + ui + 1])

            et_u = {}
            for ui, (ck0, nch) in enumerate(units):
                pool = psq if nch == NQ else psp
                lt = pool.tile([128, nch * R], f32,
                               tag="ltq" if nch == NQ else "ltp")
                for c in range(nch):
                    kc = ck0 + c
                    q, cq = kc // NQ, kc % NQ
                    for d in range(DC):
                        nc.tensor.matmul(
                            lt[:, c * R:(c + 1) * R],
                            pk(q, d * PKW + cq * 128, 128),
                            ptr_t[d],
                            start=(d == 0),
                            stop=(d == DC - 1),
                        )
                et = wpool.tile([128, nch * R], bf16,
                                tag="etq" if nch == NQ else "etp")
                nc.scalar.activation(et[:], lt[:], AF.Exp,
                                     bias=bias_t[:, 0:1], scale=inv_t)
                et_u[ui] = et
                if ui >= 1:
                    ln_op(ui - 1)
                    g_mms(ui - 1)
            ln_op(NU - 1)
            g_mms(NU - 1)

            nc.vector.tensor_copy(outt[:, :R], gacc[:])
            nc.sync.dma_start(out_d[:], outt[:])
    return nc


def _device_run(P, ohw):
    """Returns Gp0 [N,121] = E0@OHW and SUMLNE = sum_ij ln(E0+1e-5)."""
    from concourse.bass_utils import run_bass_kernel_spmd

    nc = _build_nc()
    PKW = NQ * 128
    PT = np.ascontiguousarray(P.T).astype(BF16)  # [D, N]
    ohwb = ohw.astype(BF16)                      # [N, VOCAB]
    inv_t = 1.0 / TEMP
    biasv = np.empty((128, 3), np.float32)
    biasv[:, 0] = -inv_t
    biasv[:, 1] = 1e-5
    biasv[:, 2] = 1.0

    def make_pack(kchunks):
        """kchunks: NQ global 128-k-chunk indices, in processing order."""
        pk = np.empty((128, DC * PKW + NQ * VOCAB), BF16)
        for ci, kc in enumerate(kchunks):
            for d in range(DC):
                pk[:, d * PKW + ci * 128:d * PKW + (ci + 1) * 128] = \
                    PT[d * 128:(d + 1) * 128, kc * 128:(kc + 1) * 128]
            pk[:, DC * PKW + ci * VOCAB:DC * PKW + (ci + 1) * VOCAB] = \
                ohwb[kc * 128:(kc + 1) * 128, :]
        return pk

    in_maps = []
    for c in range(NCORES):
        qc, off = c // 2, (c % 2) * 2  # own quad; chunk rotation (2 chunks)
        own = [qc * NQ + ((ci + off) % NQ) for ci in range(NQ)]
        quads = [own] + [[q * NQ + ci for ci in range(NQ)]
                         for q in range(QN) if q != qc]
        m = {f"pk{q}": make_pack(quads[q]) for q in range(QN)}
        m["biasv"] = biasv
        in_maps.append(m)
    res = run_bass_kernel_spmd(
        nc, in_maps, list(range(NCORES)),
        trace=TRACE,
        trace_cores=TRACE_CORES if TRACE else None,
    )
    globals()["LAST"] = res
    r = res.results
    Gp0 = np.concatenate(
        [np.asarray(x["outt"], np.float32)[:VOCAB, :R].T for x in r], 0)
    sumlne = float(sum(
        np.asarray(x["outt"], np.float64)[:, R:].sum() for x in r))
    return Gp0.astype(np.float64), sumlne


def _host_fallback(P, ohw):
    L = (P @ P.T - 1.0) / TEMP
    E0 = np.exp(L)
    return E0 @ ohw, float(np.log(E0 + 1e-5).sum())


def kernel(projections, targets, weights):
    P = np.asarray(projections, np.float32)
    t = np.asarray(targets).astype(np.int64)
    w = np.asarray(weights, np.float32)
    lbl = t - OFF
    tw = w[lbl].astype(np.float32)
    ohw = np.zeros((N, VOCAB), np.float32)
    ohw[np.arange(N), lbl] = tw

    try:
        Gp0, sumlne = _device_run(P, ohw)
    except Exception as e:  # pragma: no cover - safety net
        import traceback
        traceback.print_exc()
        print("DEVICE PATH FAILED - host fallback:", e)
        Gp0, sumlne = _host_fallback(P.astype(np.float64), ohw.astype(np.float64))

    twd = tw.astype(np.float64)
    s_u = np.bincount(lbl, weights=twd, minlength=VOCAB)
    Gp = Gp0 + 1e-5 * s_u[None, :]          # [N,VOCAB]
    G = twd[:, None] * Gp
    rowsumA = twd * Gp.sum(1)               # denom diagonal
    u = np.arange(VOCAB)
    Cm = np.abs(u[:, None] - u[None, :])    # [v,u]
    Bv = np.abs(u[:, None] - lbl[None, :])  # [v,j]
    LDsum = np.empty(VOCAB, np.float64)
    for v in range(VOCAB):
        M = Cm[v][None, :] >= Bv[v][:, None]  # [j,u]
        Dv = (G * M).sum(1)
        LDsum[v] = np.log(Dv).sum()
    ltw = np.log(twd)
    SLT = ltw.sum()
    num = (LDsum[lbl].sum() - np.log(rowsumA).sum()
           - sumlne - 2.0 * N * SLT
           + N * np.log1p(1e-5) + 2.0 * SLT)
    return np.float32(num / (N * (N - 1 + 1e-5)))
